# revision 36
# baseline (speedup 1.0000x reference)
"""Grouped-Query Attention kernel for Trainium2 (8 NeuronCores, SPMD).

Problem: x [4, 4096, 512] fp32, per-group Dense Q/K/V (G=4 groups of 128
features), full softmax attention within each (batch, group) pair, output
re-concatenated to [4, 4096, 512].

Sharding: B*G = 16 fully independent attention problems -> 2 per core.

v2 design (vs v1 baseline @666us):
  - 1/sqrt(gs) scale folded into Wq/bq on the host; K bias dropped
    (softmax is invariant to a per-row constant); V bias folded into the
    epilogue (rows of softmax(P) sum to 1, so O = (P V)/den + bv exactly).
  - V computed in NATURAL [t, e] layout directly (stationary = xgT chunk,
    moving = Wv) -- kills 32 PE transposes + 8 MMs per pair.
  - Q^T/K^T PSUM->SBUF evacuation moved to ScalarE (ACT is idle in the
    prologue; bias applied via ACT's free affine).
  - Attention is software-pipelined at ROUND granularity: round r issues
    score MMs of macro r interleaved (group by group) with PV/den MMs of
    macro r-1, so the PE never waits on ACT exp and stays HAM-warm.
  - exp batched as [128, 1024] ACT instructions (2 score chunks per
    instruction) to amortize ACT's 352-cycle overhead.
  - epilogue: reciprocal_approx_fast (~5x faster than DVE reciprocal),
    bias-add pre-transpose via per-partition tensor_scalar.
Compute dtype bf16 (fp32 accumulation in PSUM).
"""

import os
import sys

sys.path.insert(0, "/opt/trn_rl_repo")

import numpy as np

import concourse.bass as bass
import concourse.mybir as mybir
import concourse.tile as tile
from concourse.masks import make_identity

B, T, F, G = 4, 4096, 512, 4
GS = F // G  # 128
N_CORES = 8
PAIRS_PER_CORE = (B * G) // N_CORES  # 2
TQ = 512  # query tile width per macro (matches PSUM bank for fp32 out/den)
N_MACROS = T // TQ  # 8 per pair
N_CHUNKS = T // 128  # 32 key/time chunks
N_GROUPS = N_CHUNKS // 2  # 16 two-chunk score groups per macro
INV_SCALE = float(1.0 / (np.sqrt(np.float32(GS)) + 1e-9))

FP32 = mybir.dt.float32
BF16 = mybir.dt.bfloat16
FP8 = mybir.dt.float8e4
INT32 = mybir.dt.int32
ACTF = mybir.ActivationFunctionType
EXP_SHIFT = -4.0  # exp(s-4): keeps fp8e4m3 pt in range; cancels in normalize
# Schraudolph fast-exp2 constants: int(A*s + C) bit-viewed as fp32 ~= exp(s-4)
EXP2_A = float(2**23 * np.log2(np.e))
EXP2_C = float(2**23 * 126.94269504 + EXP_SHIFT * 2**23 * np.log2(np.e))
N_DVE_EXP = 0  # exp groups on VectorE — measured HARMFUL at every placement
# (end: delays pt for next round's PV; start: holds score-PSUM slots behind
# the prior epilogue in the DVE FIFO; mid: still 436us vs 329us). Keep 0.
DVE_EXP_G0 = 7

_NC_CACHE = None
_LAST_IN_MAPS = None


def _split_multi_waits(nc):
    """Walrus codegen rejects instructions carrying more than one semaphore
    wait on several instruction structs (DMA DIRECT2D, tensor_scalar, LDW).
    Hoist all-but-the-last wait of any multi-wait instruction onto same-engine
    NoOps inserted immediately before it: the sequencer executes them in
    order, so the gating semantics are identical."""
    n_split = 0
    for func in nc.m.functions:
        for block in func.blocks:
            new = []
            for inst in block.instructions:
                si = inst.sync_info
                waits = list(si.on_wait) if (si is not None and si.on_wait) else []
                if len(waits) > 1:
                    for w in waits[:-1]:
                        nop = mybir.InstNoOp(
                            name=nc.get_next_instruction_name(), ins=[], outs=[]
                        )
                        nop.engine = inst.engine
                        nop.sync_info = mybir.SyncInfo(on_wait=[w], on_update=[])
                        new.append(nop)
                        n_split += 1
                    inst.sync_info = mybir.SyncInfo(
                        on_wait=[waits[-1]],
                        on_update=list(si.on_update) if si.on_update else [],
                    )
                new.append(inst)
            block.instructions = new
    return n_split


def build_nc():
    nc = bass.Bass()

    ins = []
    outs = []
    for i in range(PAIRS_PER_CORE):
        ins.append(
            dict(
                x=nc.declare_dram_parameter(f"x{i}", [T, GS], FP32, isOutput=False),
                wq=nc.declare_dram_parameter(f"wq{i}", [GS, GS], FP32, isOutput=False),
                wk=nc.declare_dram_parameter(f"wk{i}", [GS, GS], FP32, isOutput=False),
                wv=nc.declare_dram_parameter(f"wv{i}", [GS, GS], FP32, isOutput=False),
                bq=nc.declare_dram_parameter(f"bq{i}", [1, GS], FP32, isOutput=False),
                bv=nc.declare_dram_parameter(f"bv{i}", [1, GS], FP32, isOutput=False),
            )
        )
        outs.append(nc.declare_dram_parameter(f"y{i}", [T, GS], FP32, isOutput=True))

    with tile.TileContext(nc) as tc:
        with (
            tc.tile_pool(name="consts", bufs=1) as consts,
            tc.tile_pool(name="wstage", bufs=2) as wstage,
            tc.tile_pool(name="xgf", bufs=2) as xgf,
            tc.tile_pool(name="xgb", bufs=2) as xgb,
            tc.tile_pool(name="big", bufs=2) as big,  # xgT/qt/kt/v_nat per pair
            tc.tile_pool(name="ptp", bufs=18) as ptp,  # exp'd prob groups
            tc.tile_pool(name="epi", bufs=2) as epi,  # epilogue sbuf tiles
            tc.tile_pool(name="ps_s", bufs=2, space="PSUM") as ps_s,  # scores
            tc.tile_pool(name="ps_o", bufs=2, space="PSUM") as ps_o,  # out^T
            tc.tile_pool(name="ps_d", bufs=2, space="PSUM") as ps_d,  # denom
        ):
            ident_bf = consts.tile([128, 128], BF16)
            make_identity(nc, ident_bf)
            ones_f8 = consts.tile([128, 2, 128], FP8)
            nc.vector.memset(ones_f8, 1.0)
            exp_bias = consts.tile([128, 1], FP32)
            nc.vector.memset(exp_bias, EXP_SHIFT)
            # preload the exp activation table during the prologue so the
            # first real exp doesn't pay the ~2.7us ACT_TABLE_LOAD
            warm = consts.tile([128, 1], FP32)
            nc.scalar.activation(warm, exp_bias, ACTF.Exp)

            w_bf = []  # per pair dict
            b_col = []
            xgT = []
            qt = []
            kt = []
            v_nat = []

            def prologue_a(i):
                """x load + transposes + Q^T/K^T, interleaved per 4-chunk slice
                so kt/qt slices land progressively and round-i scores (and
                ACT exp) can start as early as possible."""
                p = ins[i]
                wd = {}
                for nm in ("wq", "wk", "wv"):
                    wf = wstage.tile([128, 128], FP32, tag="wf")
                    nc.gpsimd.dma_start(out=wf, in_=p[nm][:, :])
                    wb = consts.tile([128, 128], BF16, tag=f"{nm}{i}")
                    nc.vector.tensor_copy(wb, wf)
                    wd[nm] = wb
                w_bf.append(wd)
                bd = {}
                for nm in ("bq", "bv"):
                    bc = consts.tile([128, 1], FP32, tag=f"{nm}{i}")
                    nc.gpsimd.dma_start(out=bc, in_=p[nm][:, :].rearrange("o d -> d o"))
                    bd[nm] = bc
                b_col.append(bd)

                xg_f = xgf.tile([128, N_CHUNKS, 128], FP32, tag="xg_f")
                xg_b = xgb.tile([128, N_CHUNKS, 128], BF16, tag="xg_b")
                xt = big.tile([128, T], BF16, tag="xgT")
                qtt = big.tile([128, T], BF16, tag="qt")
                ktt = big.tile([128, T], BF16, tag="kt")
                for q in range(8):
                    qsl_c = slice(q * 4, (q + 1) * 4)
                    nc.gpsimd.dma_start(
                        out=xg_f[:, qsl_c, :],
                        in_=p["x"][q * 512 : (q + 1) * 512, :].rearrange(
                            "(c p) d -> p c d", p=128
                        ),
                    )
                    nc.vector.tensor_copy(xg_b[:, qsl_c, :], xg_f[:, qsl_c, :])
                    for c in range(q * 4, (q + 1) * 4):
                        pst = ps_s.tile([128, 128], BF16, tag="sc")
                        nc.tensor.transpose(pst, xg_b[:, c, :], ident_bf)
                        nc.vector.tensor_copy(xt[:, c * 128 : (c + 1) * 128], pst)
                    sl = slice(q * 512, (q + 1) * 512)
                    psq = ps_o.tile([128, 512], FP32, tag="out")
                    nc.tensor.matmul(psq, wd["wq"], xt[:, sl], start=True, stop=True)
                    nc.scalar.activation(qtt[:, sl], psq, ACTF.Identity, bias=bd["bq"])
                    psk = ps_o.tile([128, 512], FP32, tag="out")
                    nc.tensor.matmul(psk, wd["wk"], xt[:, sl], start=True, stop=True)
                    nc.scalar.activation(ktt[:, sl], psk, ACTF.Copy)
                xgT.append(xt)
                qt.append(qtt)
                kt.append(ktt)

            def prologue_b(i):
                # V natural [t, e] fp8: stationary = xgT chunk, moving = Wv
                vn = big.tile([128, N_CHUNKS, 128], FP8, tag="v_nat")
                for c in range(N_CHUNKS):
                    psv = ps_d.tile([128, 128], FP32, tag="den")
                    nc.tensor.matmul(
                        psv, xgT[i][:, c * 128 : (c + 1) * 128], w_bf[i]["wv"],
                        start=True, stop=True,
                    )
                    nc.vector.tensor_copy(vn[:, c, :], psv)
                v_nat.append(vn)

            prologue_a(0)

            # ---------------- attention rounds (software-pipelined) ----------
            N_TOTAL = PAIRS_PER_CORE * N_MACROS  # 16 macros across both pairs
            pt_prev = None
            for r in range(N_TOTAL + 1):
                if r == 1:
                    # V of pair 0 (needed now), then pair-1 prologue — all of
                    # it overlaps round 0's ACT exp work
                    prologue_b(0)
                    prologue_a(1)
                if r == 4:
                    # pair-1 V is only needed from round 9; emitting it here
                    # spreads the prologue PE hump across more ACT work
                    prologue_b(1)
                pt_cur = [None] * N_GROUPS
                if r > 0:
                    p2, m2 = divmod(r - 1, N_MACROS)
                    ps_out = ps_o.tile([128, TQ], FP32, tag="out")
                    ps_den = ps_d.tile([128, TQ], FP32, tag="den")
                if r < N_TOTAL:
                    p1, m1 = divmod(r, N_MACROS)
                    qsl = qt[p1][:, m1 * TQ : (m1 + 1) * TQ]
                for g in range(N_GROUPS):
                    if r < N_TOTAL:
                        c0, c1 = 2 * g, 2 * g + 1
                        sc = ps_s.tile([128, 2, 512], FP32, tag="sc")
                        nc.tensor.matmul(
                            sc[:, 0, :],
                            kt[p1][:, c0 * 128 : (c0 + 1) * 128],
                            qsl, start=True, stop=True,
                        )
                        nc.tensor.matmul(
                            sc[:, 1, :],
                            kt[p1][:, c1 * 128 : (c1 + 1) * 128],
                            qsl, start=True, stop=True,
                        )
                        ptg = ptp.tile([128, 2, 512], FP8, tag="pt")
                        if DVE_EXP_G0 <= g < DVE_EXP_G0 + N_DVE_EXP:
                            # fast-exp2 on VectorE: bits = int(A*s + C), viewed fp32
                            ti = epi.tile([128, 2, 512], INT32, tag="ti")
                            nc.vector.tensor_scalar(
                                ti, sc, EXP2_A, EXP2_C,
                                mybir.AluOpType.mult, mybir.AluOpType.add,
                            )
                            nc.vector.tensor_copy(ptg, ti[:, :, :].bitcast(FP32))
                        else:
                            nc.scalar.activation(ptg, sc, ACTF.Exp, bias=exp_bias)
                        pt_cur[g] = ptg
                    if 0 < r < N_TOTAL:
                        first, last = g == 0, g == N_GROUPS - 1
                        nc.tensor.matmul(
                            ps_out,
                            v_nat[p2][:, 2 * g : 2 * g + 2, :],
                            pt_prev[g],
                            start=first, stop=last,
                            perf_mode=mybir.MatmulPerfMode.DoubleRow,
                        )
                        nc.tensor.matmul(
                            ps_den, ones_f8, pt_prev[g],
                            start=first, stop=last,
                            perf_mode=mybir.MatmulPerfMode.DoubleRow,
                        )
                if r == N_TOTAL:
                    # final round: all den MMs first so the reciprocal (the
                    # long pole of the exposed tail) overlaps the PV MMs
                    for g in range(N_GROUPS):
                        nc.tensor.matmul(
                            ps_den, ones_f8, pt_prev[g],
                            start=g == 0, stop=g == N_GROUPS - 1,
                            perf_mode=mybir.MatmulPerfMode.DoubleRow,
                        )
                    for g in range(N_GROUPS):
                        nc.tensor.matmul(
                            ps_out,
                            v_nat[p2][:, 2 * g : 2 * g + 2, :],
                            pt_prev[g],
                            start=g == 0, stop=g == N_GROUPS - 1,
                            perf_mode=mybir.MatmulPerfMode.DoubleRow,
                        )
                # epilogue of macro r-1
                if r > 0:
                    rec = epi.tile([128, TQ], FP32, tag="rec")
                    if r >= N_TOTAL - 1:
                        # tail rounds: ACT is idle there; 1/den = exp(-ln(den))
                        # keeps the slow DVE reciprocal off the exposed tail
                        dln = epi.tile([128, TQ], FP32, tag="dln")
                        nc.scalar.activation(dln, ps_den, ACTF.Ln)
                        nc.scalar.activation(rec, dln, ACTF.Exp, scale=-1.0)
                    else:
                        nc.vector.reciprocal(rec, ps_den)
                    onorm = epi.tile([128, TQ], BF16, tag="onorm")
                    nc.vector.tensor_mul(onorm, ps_out, rec)
                    nc.vector.tensor_scalar_add(onorm, onorm, b_col[p2]["bv"])
                    ofin = epi.tile([128, TQ // 128, 128], FP32, tag="ofin")
                    if r >= N_TOTAL - 1:
                        # tail rounds: PE is idle — transpose there instead of
                        # queueing 4 serialized ~1.2us xbar transposes
                        for j in range(TQ // 128):
                            pst = ps_s.tile([128, 128], BF16, tag="sc")
                            nc.tensor.transpose(
                                pst, onorm[:, j * 128 : (j + 1) * 128], ident_bf
                            )
                            nc.vector.tensor_copy(ofin[:, j, :], pst)
                    else:
                        onat = epi.tile([128, TQ // 128, 128], BF16, tag="onat")
                        for j in range(TQ // 128):
                            nc.sync.dma_start_transpose(
                                onat[:, j, :], onorm[:, j * 128 : (j + 1) * 128]
                            )
                        nc.vector.tensor_copy(ofin, onat)
                    nc.gpsimd.dma_start(
                        out=outs[p2][m2 * TQ : (m2 + 1) * TQ, :].rearrange(
                            "(c p) d -> p c d", p=128
                        ),
                        in_=ofin,
                    )
                pt_prev = pt_cur
    _split_multi_waits(nc)
    return nc


def _get_nc():
    global _NC_CACHE
    if _NC_CACHE is None:
        _NC_CACHE = build_nc()
    return _NC_CACHE


def kernel(**inputs: np.ndarray) -> np.ndarray:
    x = np.ascontiguousarray(inputs["x"], dtype=np.float32)
    Wq = np.asarray(inputs["Wq"], dtype=np.float32) * INV_SCALE
    Wk = np.asarray(inputs["Wk"], dtype=np.float32)
    Wv = np.asarray(inputs["Wv"], dtype=np.float32)
    bq = np.asarray(inputs["bq"], dtype=np.float32) * INV_SCALE
    bv = np.asarray(inputs["bv"], dtype=np.float32)

    nc = _get_nc()

    in_maps = []
    for core in range(N_CORES):
        m = {}
        for i in range(PAIRS_PER_CORE):
            pair = core * PAIRS_PER_CORE + i
            b, g = pair // G, pair % G
            sl = slice(g * GS, (g + 1) * GS)
            m[f"x{i}"] = np.ascontiguousarray(x[b, :, sl])
            m[f"wq{i}"] = np.ascontiguousarray(Wq[g])
            m[f"wk{i}"] = np.ascontiguousarray(Wk[g])
            m[f"wv{i}"] = np.ascontiguousarray(Wv[g])
            m[f"bq{i}"] = np.ascontiguousarray(bq[g].reshape(1, GS))
            m[f"bv{i}"] = np.ascontiguousarray(bv[g].reshape(1, GS))
        in_maps.append(m)

    global _LAST_IN_MAPS
    _LAST_IN_MAPS = in_maps

    from concourse.bass_utils import run_bass_kernel_spmd

    res = run_bass_kernel_spmd(nc, in_maps, list(range(N_CORES)))

    y = np.empty((B, T, F), dtype=np.float32)
    for core in range(N_CORES):
        for i in range(PAIRS_PER_CORE):
            pair = core * PAIRS_PER_CORE + i
            b, g = pair // G, pair % G
            y[b, :, g * GS : (g + 1) * GS] = res.results[core][f"y{i}"]
    return y


# revision 38
# speedup vs baseline: 1.0199x; 1.0199x over previous
"""Grouped-Query Attention kernel for Trainium2 (8 NeuronCores, SPMD).

Problem: x [4, 4096, 512] fp32, per-group Dense Q/K/V (G=4 groups of 128
features), full softmax attention within each (batch, group) pair, output
re-concatenated to [4, 4096, 512].

Sharding: B*G = 16 fully independent attention problems -> 2 per core.

v2 design (vs v1 baseline @666us):
  - 1/sqrt(gs) scale folded into Wq/bq on the host; K bias dropped
    (softmax is invariant to a per-row constant); V bias folded into the
    epilogue (rows of softmax(P) sum to 1, so O = (P V)/den + bv exactly).
  - V computed in NATURAL [t, e] layout directly (stationary = xgT chunk,
    moving = Wv) -- kills 32 PE transposes + 8 MMs per pair.
  - Q^T/K^T PSUM->SBUF evacuation moved to ScalarE (ACT is idle in the
    prologue; bias applied via ACT's free affine).
  - Attention is software-pipelined at ROUND granularity: round r issues
    score MMs of macro r interleaved (group by group) with PV/den MMs of
    macro r-1, so the PE never waits on ACT exp and stays HAM-warm.
  - exp batched as [128, 1024] ACT instructions (2 score chunks per
    instruction) to amortize ACT's 352-cycle overhead.
  - epilogue: reciprocal_approx_fast (~5x faster than DVE reciprocal),
    bias-add pre-transpose via per-partition tensor_scalar.
Compute dtype bf16 (fp32 accumulation in PSUM).
"""

import os
import sys

sys.path.insert(0, "/opt/trn_rl_repo")

import numpy as np

import concourse.bass as bass
import concourse.mybir as mybir
import concourse.tile as tile
from concourse.masks import make_identity

B, T, F, G = 4, 4096, 512, 4
GS = F // G  # 128
N_CORES = 8
PAIRS_PER_CORE = (B * G) // N_CORES  # 2
TQ = 512  # query tile width per macro (matches PSUM bank for fp32 out/den)
N_MACROS = T // TQ  # 8 per pair
N_CHUNKS = T // 128  # 32 key/time chunks
N_GROUPS = N_CHUNKS // 2  # 16 two-chunk score groups per macro
INV_SCALE = float(1.0 / (np.sqrt(np.float32(GS)) + 1e-9))

FP32 = mybir.dt.float32
BF16 = mybir.dt.bfloat16
FP8 = mybir.dt.float8e4
INT32 = mybir.dt.int32
ACTF = mybir.ActivationFunctionType
EXP_SHIFT = -4.0  # exp(s-4): keeps fp8e4m3 pt in range; cancels in normalize
# Schraudolph fast-exp2 constants: int(A*s + C) bit-viewed as fp32 ~= exp(s-4)
EXP2_A = float(2**23 * np.log2(np.e))
EXP2_C = float(2**23 * 126.94269504 + EXP_SHIFT * 2**23 * np.log2(np.e))
N_DVE_EXP = 0  # exp groups on VectorE — measured HARMFUL at every placement
# (end: delays pt for next round's PV; start: holds score-PSUM slots behind
# the prior epilogue in the DVE FIFO; mid: still 436us vs 329us). Keep 0.
DVE_EXP_G0 = 7

_NC_CACHE = None
_LAST_IN_MAPS = None


def _split_multi_waits(nc):
    """Walrus codegen rejects instructions carrying more than one semaphore
    wait on several instruction structs (DMA DIRECT2D, tensor_scalar, LDW).
    Hoist all-but-the-last wait of any multi-wait instruction onto same-engine
    NoOps inserted immediately before it: the sequencer executes them in
    order, so the gating semantics are identical."""
    n_split = 0
    for func in nc.m.functions:
        for block in func.blocks:
            new = []
            for inst in block.instructions:
                si = inst.sync_info
                waits = list(si.on_wait) if (si is not None and si.on_wait) else []
                if len(waits) > 1:
                    for w in waits[:-1]:
                        nop = mybir.InstNoOp(
                            name=nc.get_next_instruction_name(), ins=[], outs=[]
                        )
                        nop.engine = inst.engine
                        nop.sync_info = mybir.SyncInfo(on_wait=[w], on_update=[])
                        new.append(nop)
                        n_split += 1
                    inst.sync_info = mybir.SyncInfo(
                        on_wait=[waits[-1]],
                        on_update=list(si.on_update) if si.on_update else [],
                    )
                new.append(inst)
            block.instructions = new
    return n_split


def build_nc():
    nc = bass.Bass()

    ins = []
    outs = []
    for i in range(PAIRS_PER_CORE):
        ins.append(
            dict(
                x=nc.declare_dram_parameter(f"x{i}", [T, GS], FP32, isOutput=False),
                wq=nc.declare_dram_parameter(f"wq{i}", [GS, GS], FP32, isOutput=False),
                wk=nc.declare_dram_parameter(f"wk{i}", [GS, GS], FP32, isOutput=False),
                wv=nc.declare_dram_parameter(f"wv{i}", [GS, GS], FP32, isOutput=False),
                bq=nc.declare_dram_parameter(f"bq{i}", [1, GS], FP32, isOutput=False),
                bv=nc.declare_dram_parameter(f"bv{i}", [1, GS], FP32, isOutput=False),
            )
        )
        outs.append(nc.declare_dram_parameter(f"y{i}", [T, GS], FP32, isOutput=True))

    with tile.TileContext(nc) as tc:
        with (
            tc.tile_pool(name="consts", bufs=1) as consts,
            tc.tile_pool(name="wstage", bufs=2) as wstage,
            tc.tile_pool(name="xgf", bufs=2) as xgf,
            tc.tile_pool(name="xgb", bufs=2) as xgb,
            tc.tile_pool(name="big", bufs=2) as big,  # xgT/qt/kt/v_nat per pair
            tc.tile_pool(name="ptp", bufs=18) as ptp,  # exp'd prob groups
            tc.tile_pool(name="epi", bufs=2) as epi,  # epilogue sbuf tiles
            tc.tile_pool(name="ps_s", bufs=2, space="PSUM") as ps_s,  # scores
            tc.tile_pool(name="ps_o", bufs=2, space="PSUM") as ps_o,  # out^T
            tc.tile_pool(name="ps_d", bufs=2, space="PSUM") as ps_d,  # denom
        ):
            ident_bf = consts.tile([128, 128], BF16)
            make_identity(nc, ident_bf)
            ones_f8 = consts.tile([128, 2, 128], FP8)
            nc.vector.memset(ones_f8, 1.0)
            exp_bias = consts.tile([128, 1], FP32)
            nc.vector.memset(exp_bias, EXP_SHIFT)
            # preload the exp activation table during the prologue so the
            # first real exp doesn't pay the ~2.7us ACT_TABLE_LOAD
            warm = consts.tile([128, 1], FP32)
            nc.scalar.activation(warm, exp_bias, ACTF.Exp)

            w_bf = []  # per pair dict
            b_col = []
            xgT = []
            qt = []
            kt = []
            v_nat = []

            def prologue_a(i):
                """x load + transposes + Q^T/K^T, interleaved per 4-chunk slice
                so kt/qt slices land progressively and round-i scores (and
                ACT exp) can start as early as possible."""
                p = ins[i]
                wd = {}
                for nm in ("wq", "wk", "wv"):
                    wf = wstage.tile([128, 128], FP32, tag="wf")
                    nc.gpsimd.dma_start(out=wf, in_=p[nm][:, :])
                    wb = consts.tile([128, 128], BF16, tag=f"{nm}{i}")
                    nc.vector.tensor_copy(wb, wf)
                    wd[nm] = wb
                w_bf.append(wd)
                bd = {}
                for nm in ("bq", "bv"):
                    bc = consts.tile([128, 1], FP32, tag=f"{nm}{i}")
                    nc.gpsimd.dma_start(out=bc, in_=p[nm][:, :].rearrange("o d -> d o"))
                    bd[nm] = bc
                b_col.append(bd)

                xg_f = xgf.tile([128, N_CHUNKS, 128], FP32, tag="xg_f")
                xg_b = xgb.tile([128, N_CHUNKS, 128], BF16, tag="xg_b")
                xt = big.tile([128, T], BF16, tag="xgT")
                qtt = big.tile([128, T], BF16, tag="qt")
                ktt = big.tile([128, T], BF16, tag="kt")
                for q in range(8):
                    qsl_c = slice(q * 4, (q + 1) * 4)
                    nc.gpsimd.dma_start(
                        out=xg_f[:, qsl_c, :],
                        in_=p["x"][q * 512 : (q + 1) * 512, :].rearrange(
                            "(c p) d -> p c d", p=128
                        ),
                    )
                    nc.vector.tensor_copy(xg_b[:, qsl_c, :], xg_f[:, qsl_c, :])
                    for c in range(q * 4, (q + 1) * 4):
                        pst = ps_s.tile([128, 128], BF16, tag="sc")
                        nc.tensor.transpose(pst, xg_b[:, c, :], ident_bf)
                        nc.vector.tensor_copy(xt[:, c * 128 : (c + 1) * 128], pst)
                    sl = slice(q * 512, (q + 1) * 512)
                    psq = ps_o.tile([128, 512], FP32, tag="out")
                    nc.tensor.matmul(psq, wd["wq"], xt[:, sl], start=True, stop=True)
                    nc.scalar.activation(qtt[:, sl], psq, ACTF.Identity, bias=bd["bq"])
                    psk = ps_o.tile([128, 512], FP32, tag="out")
                    nc.tensor.matmul(psk, wd["wk"], xt[:, sl], start=True, stop=True)
                    nc.scalar.activation(ktt[:, sl], psk, ACTF.Copy)
                xgT.append(xt)
                qt.append(qtt)
                kt.append(ktt)

            def prologue_b(i):
                # V natural [t, e] fp8: stationary = xgT chunk, moving = Wv
                vn = big.tile([128, N_CHUNKS, 128], FP8, tag="v_nat")
                for c in range(N_CHUNKS):
                    psv = ps_d.tile([128, 128], FP32, tag="den")
                    nc.tensor.matmul(
                        psv, xgT[i][:, c * 128 : (c + 1) * 128], w_bf[i]["wv"],
                        start=True, stop=True,
                    )
                    nc.vector.tensor_copy(vn[:, c, :], psv)
                v_nat.append(vn)

            prologue_a(0)

            # ---------------- attention rounds (software-pipelined) ----------
            N_TOTAL = PAIRS_PER_CORE * N_MACROS  # 16 macros across both pairs
            pt_prev = None
            for r in range(N_TOTAL + 1):
                if r == 1:
                    # V of pair 0 (needed now), then pair-1 prologue — all of
                    # it overlaps round 0's ACT exp work
                    prologue_b(0)
                    prologue_a(1)
                    prologue_b(1)
                pt_cur = [None] * N_GROUPS
                if r > 0:
                    p2, m2 = divmod(r - 1, N_MACROS)
                    ps_out = ps_o.tile([128, TQ], FP32, tag="out")
                    ps_den = ps_d.tile([128, TQ], FP32, tag="den")
                if r < N_TOTAL:
                    p1, m1 = divmod(r, N_MACROS)
                    qsl = qt[p1][:, m1 * TQ : (m1 + 1) * TQ]
                for g in range(N_GROUPS):
                    if r < N_TOTAL:
                        c0, c1 = 2 * g, 2 * g + 1
                        sc = ps_s.tile([128, 2, 512], FP32, tag="sc")
                        nc.tensor.matmul(
                            sc[:, 0, :],
                            kt[p1][:, c0 * 128 : (c0 + 1) * 128],
                            qsl, start=True, stop=True,
                        )
                        nc.tensor.matmul(
                            sc[:, 1, :],
                            kt[p1][:, c1 * 128 : (c1 + 1) * 128],
                            qsl, start=True, stop=True,
                        )
                        ptg = ptp.tile([128, 2, 512], FP8, tag="pt")
                        if DVE_EXP_G0 <= g < DVE_EXP_G0 + N_DVE_EXP:
                            # fast-exp2 on VectorE: bits = int(A*s + C), viewed fp32
                            ti = epi.tile([128, 2, 512], INT32, tag="ti")
                            nc.vector.tensor_scalar(
                                ti, sc, EXP2_A, EXP2_C,
                                mybir.AluOpType.mult, mybir.AluOpType.add,
                            )
                            nc.vector.tensor_copy(ptg, ti[:, :, :].bitcast(FP32))
                        else:
                            nc.scalar.activation(ptg, sc, ACTF.Exp, bias=exp_bias)
                        pt_cur[g] = ptg
                    if 0 < r < N_TOTAL:
                        first, last = g == 0, g == N_GROUPS - 1
                        nc.tensor.matmul(
                            ps_out,
                            v_nat[p2][:, 2 * g : 2 * g + 2, :],
                            pt_prev[g],
                            start=first, stop=last,
                            perf_mode=mybir.MatmulPerfMode.DoubleRow,
                        )
                        nc.tensor.matmul(
                            ps_den, ones_f8, pt_prev[g],
                            start=first, stop=last,
                            perf_mode=mybir.MatmulPerfMode.DoubleRow,
                        )
                if r == N_TOTAL:
                    # final round: all den MMs first so the reciprocal (the
                    # long pole of the exposed tail) overlaps the PV MMs
                    for g in range(N_GROUPS):
                        nc.tensor.matmul(
                            ps_den, ones_f8, pt_prev[g],
                            start=g == 0, stop=g == N_GROUPS - 1,
                            perf_mode=mybir.MatmulPerfMode.DoubleRow,
                        )
                    for g in range(N_GROUPS):
                        nc.tensor.matmul(
                            ps_out,
                            v_nat[p2][:, 2 * g : 2 * g + 2, :],
                            pt_prev[g],
                            start=g == 0, stop=g == N_GROUPS - 1,
                            perf_mode=mybir.MatmulPerfMode.DoubleRow,
                        )
                # epilogue of macro r-1
                if r > 0:
                    rec = epi.tile([128, TQ], FP32, tag="rec")
                    if r >= N_TOTAL - 1:
                        # tail rounds: ACT is idle there; 1/den = exp(-ln(den))
                        # keeps the slow DVE reciprocal off the exposed tail
                        dln = epi.tile([128, TQ], FP32, tag="dln")
                        nc.scalar.activation(dln, ps_den, ACTF.Ln)
                        nc.scalar.activation(rec, dln, ACTF.Exp, scale=-1.0)
                    else:
                        nc.vector.reciprocal(rec, ps_den)
                    onorm = epi.tile([128, TQ], BF16, tag="onorm")
                    nc.vector.tensor_mul(onorm, ps_out, rec)
                    nc.vector.tensor_scalar_add(onorm, onorm, b_col[p2]["bv"])
                    onat = epi.tile([128, TQ // 128, 128], BF16, tag="onat")
                    for j in range(TQ // 128):
                        nc.sync.dma_start_transpose(
                            onat[:, j, :], onorm[:, j * 128 : (j + 1) * 128]
                        )
                    ofin = epi.tile([128, TQ // 128, 128], FP32, tag="ofin")
                    nc.vector.tensor_copy(ofin, onat)
                    nc.gpsimd.dma_start(
                        out=outs[p2][m2 * TQ : (m2 + 1) * TQ, :].rearrange(
                            "(c p) d -> p c d", p=128
                        ),
                        in_=ofin,
                    )
                pt_prev = pt_cur
    _split_multi_waits(nc)
    return nc


def _get_nc():
    global _NC_CACHE
    if _NC_CACHE is None:
        _NC_CACHE = build_nc()
    return _NC_CACHE


def kernel(**inputs: np.ndarray) -> np.ndarray:
    x = np.ascontiguousarray(inputs["x"], dtype=np.float32)
    Wq = np.asarray(inputs["Wq"], dtype=np.float32) * INV_SCALE
    Wk = np.asarray(inputs["Wk"], dtype=np.float32)
    Wv = np.asarray(inputs["Wv"], dtype=np.float32)
    bq = np.asarray(inputs["bq"], dtype=np.float32) * INV_SCALE
    bv = np.asarray(inputs["bv"], dtype=np.float32)

    nc = _get_nc()

    in_maps = []
    for core in range(N_CORES):
        m = {}
        for i in range(PAIRS_PER_CORE):
            pair = core * PAIRS_PER_CORE + i
            b, g = pair // G, pair % G
            sl = slice(g * GS, (g + 1) * GS)
            m[f"x{i}"] = np.ascontiguousarray(x[b, :, sl])
            m[f"wq{i}"] = np.ascontiguousarray(Wq[g])
            m[f"wk{i}"] = np.ascontiguousarray(Wk[g])
            m[f"wv{i}"] = np.ascontiguousarray(Wv[g])
            m[f"bq{i}"] = np.ascontiguousarray(bq[g].reshape(1, GS))
            m[f"bv{i}"] = np.ascontiguousarray(bv[g].reshape(1, GS))
        in_maps.append(m)

    global _LAST_IN_MAPS
    _LAST_IN_MAPS = in_maps

    from concourse.bass_utils import run_bass_kernel_spmd

    res = run_bass_kernel_spmd(nc, in_maps, list(range(N_CORES)))

    y = np.empty((B, T, F), dtype=np.float32)
    for core in range(N_CORES):
        for i in range(PAIRS_PER_CORE):
            pair = core * PAIRS_PER_CORE + i
            b, g = pair // G, pair % G
            y[b, :, g * GS : (g + 1) * GS] = res.results[core][f"y{i}"]
    return y


# revision 40
# speedup vs baseline: 1.0535x; 1.0329x over previous
"""Grouped-Query Attention kernel for Trainium2 (8 NeuronCores, SPMD).

Problem: x [4, 4096, 512] fp32, per-group Dense Q/K/V (G=4 groups of 128
features), full softmax attention within each (batch, group) pair, output
re-concatenated to [4, 4096, 512].

Sharding: B*G = 16 fully independent attention problems -> 2 per core.

v2 design (vs v1 baseline @666us):
  - 1/sqrt(gs) scale folded into Wq/bq on the host; K bias dropped
    (softmax is invariant to a per-row constant); V bias folded into the
    epilogue (rows of softmax(P) sum to 1, so O = (P V)/den + bv exactly).
  - V computed in NATURAL [t, e] layout directly (stationary = xgT chunk,
    moving = Wv) -- kills 32 PE transposes + 8 MMs per pair.
  - Q^T/K^T PSUM->SBUF evacuation moved to ScalarE (ACT is idle in the
    prologue; bias applied via ACT's free affine).
  - Attention is software-pipelined at ROUND granularity: round r issues
    score MMs of macro r interleaved (group by group) with PV/den MMs of
    macro r-1, so the PE never waits on ACT exp and stays HAM-warm.
  - exp batched as [128, 1024] ACT instructions (2 score chunks per
    instruction) to amortize ACT's 352-cycle overhead.
  - epilogue: reciprocal_approx_fast (~5x faster than DVE reciprocal),
    bias-add pre-transpose via per-partition tensor_scalar.
Compute dtype bf16 (fp32 accumulation in PSUM).
"""

import os
import sys

sys.path.insert(0, "/opt/trn_rl_repo")

import numpy as np

import concourse.bass as bass
import concourse.mybir as mybir
import concourse.tile as tile
from concourse.masks import make_identity

B, T, F, G = 4, 4096, 512, 4
GS = F // G  # 128
N_CORES = 8
PAIRS_PER_CORE = (B * G) // N_CORES  # 2
TQ = 512  # query tile width per macro (matches PSUM bank for fp32 out/den)
N_MACROS = T // TQ  # 8 per pair
N_CHUNKS = T // 128  # 32 key/time chunks
N_GROUPS = N_CHUNKS // 2  # 16 two-chunk score groups per macro
INV_SCALE = float(1.0 / (np.sqrt(np.float32(GS)) + 1e-9))

FP32 = mybir.dt.float32
BF16 = mybir.dt.bfloat16
FP8 = mybir.dt.float8e4
INT32 = mybir.dt.int32
ACTF = mybir.ActivationFunctionType
EXP_SHIFT = -4.0  # exp(s-4): keeps fp8e4m3 pt in range; cancels in normalize
# Schraudolph fast-exp2 constants: int(A*s + C) bit-viewed as fp32 ~= exp(s-4)
EXP2_A = float(2**23 * np.log2(np.e))
EXP2_C = float(2**23 * 126.94269504 + EXP_SHIFT * 2**23 * np.log2(np.e))
N_DVE_EXP = 0  # exp groups on VectorE — measured HARMFUL at every placement
# (end: delays pt for next round's PV; start: holds score-PSUM slots behind
# the prior epilogue in the DVE FIFO; mid: still 436us vs 329us). Keep 0.
DVE_EXP_G0 = 7

_NC_CACHE = None
_LAST_IN_MAPS = None


def _split_multi_waits(nc):
    """Walrus codegen rejects instructions carrying more than one semaphore
    wait on several instruction structs (DMA DIRECT2D, tensor_scalar, LDW).
    Hoist all-but-the-last wait of any multi-wait instruction onto same-engine
    NoOps inserted immediately before it: the sequencer executes them in
    order, so the gating semantics are identical."""
    n_split = 0
    for func in nc.m.functions:
        for block in func.blocks:
            new = []
            for inst in block.instructions:
                si = inst.sync_info
                waits = list(si.on_wait) if (si is not None and si.on_wait) else []
                if len(waits) > 1:
                    for w in waits[:-1]:
                        nop = mybir.InstNoOp(
                            name=nc.get_next_instruction_name(), ins=[], outs=[]
                        )
                        nop.engine = inst.engine
                        nop.sync_info = mybir.SyncInfo(on_wait=[w], on_update=[])
                        new.append(nop)
                        n_split += 1
                    inst.sync_info = mybir.SyncInfo(
                        on_wait=[waits[-1]],
                        on_update=list(si.on_update) if si.on_update else [],
                    )
                new.append(inst)
            block.instructions = new
    return n_split


def build_nc():
    nc = bass.Bass()

    ins = []
    outs = []
    for i in range(PAIRS_PER_CORE):
        ins.append(
            dict(
                x=nc.declare_dram_parameter(f"x{i}", [T, GS], FP32, isOutput=False),
                wq=nc.declare_dram_parameter(f"wq{i}", [GS, GS], FP32, isOutput=False),
                wk=nc.declare_dram_parameter(f"wk{i}", [GS, GS], FP32, isOutput=False),
                wv=nc.declare_dram_parameter(f"wv{i}", [GS, GS], FP32, isOutput=False),
                bq=nc.declare_dram_parameter(f"bq{i}", [1, GS], FP32, isOutput=False),
                bv=nc.declare_dram_parameter(f"bv{i}", [1, GS], FP32, isOutput=False),
            )
        )
        outs.append(nc.declare_dram_parameter(f"y{i}", [T, GS], FP32, isOutput=True))

    with tile.TileContext(nc) as tc:
        with (
            tc.tile_pool(name="consts", bufs=1) as consts,
            tc.tile_pool(name="wstage", bufs=2) as wstage,
            tc.tile_pool(name="xgf", bufs=2) as xgf,
            tc.tile_pool(name="xgb", bufs=2) as xgb,
            tc.tile_pool(name="big", bufs=2) as big,  # xgT/qt/kt/v_nat per pair
            tc.tile_pool(name="ptp", bufs=18) as ptp,  # exp'd prob groups
            tc.tile_pool(name="epi", bufs=2) as epi,  # epilogue sbuf tiles
            tc.tile_pool(name="ps_s", bufs=2, space="PSUM") as ps_s,  # scores
            tc.tile_pool(name="ps_o", bufs=2, space="PSUM") as ps_o,  # out^T
            tc.tile_pool(name="ps_d", bufs=2, space="PSUM") as ps_d,  # denom
        ):
            ident_bf = consts.tile([128, 128], BF16)
            make_identity(nc, ident_bf)
            ones_f8 = consts.tile([128, 2, 128], FP8)
            nc.vector.memset(ones_f8, 1.0)
            exp_bias = consts.tile([128, 1], FP32)
            nc.vector.memset(exp_bias, EXP_SHIFT)
            # preload the exp activation table during the prologue so the
            # first real exp doesn't pay the ~2.7us ACT_TABLE_LOAD
            warm = consts.tile([128, 1], FP32)
            nc.scalar.activation(warm, exp_bias, ACTF.Exp)

            w_bf = []  # per pair dict
            b_col = []
            xgT = []
            qt = []
            kt = []
            v_nat = []

            def prologue_a(i):
                """x load + transposes + Q^T/K^T, interleaved per 4-chunk slice
                so kt/qt slices land progressively and round-i scores (and
                ACT exp) can start as early as possible."""
                p = ins[i]
                xg_f = xgf.tile([128, N_CHUNKS, 128], FP32, tag="xg_f")
                xg_b = xgb.tile([128, N_CHUNKS, 128], BF16, tag="xg_b")
                xt = big.tile([128, T], BF16, tag="xgT")
                qtt = big.tile([128, T], BF16, tag="qt")
                ktt = big.tile([128, T], BF16, tag="kt")

                def load_piece(q):
                    nc.gpsimd.dma_start(
                        out=xg_f[:, q * 4 : (q + 1) * 4, :],
                        in_=p["x"][q * 512 : (q + 1) * 512, :].rearrange(
                            "(c p) d -> p c d", p=128
                        ),
                    )

                # x piece 0 ahead of the weight DMAs on the SWDGE queue — it
                # gates the whole pipeline; the weights aren't needed until
                # the first QK matmul ~2us later
                load_piece(0)
                wd = {}
                for nm in ("wq", "wk", "wv"):
                    wf = wstage.tile([128, 128], FP32, tag="wf")
                    nc.gpsimd.dma_start(out=wf, in_=p[nm][:, :])
                    wb = consts.tile([128, 128], BF16, tag=f"{nm}{i}")
                    nc.vector.tensor_copy(wb, wf)
                    wd[nm] = wb
                w_bf.append(wd)
                bd = {}
                for nm in ("bq", "bv"):
                    bc = consts.tile([128, 1], FP32, tag=f"{nm}{i}")
                    nc.gpsimd.dma_start(out=bc, in_=p[nm][:, :].rearrange("o d -> d o"))
                    bd[nm] = bc
                b_col.append(bd)

                for q in range(8):
                    qsl_c = slice(q * 4, (q + 1) * 4)
                    if q + 1 < 8:
                        load_piece(q + 1)
                    nc.vector.tensor_copy(xg_b[:, qsl_c, :], xg_f[:, qsl_c, :])
                    for c in range(q * 4, (q + 1) * 4):
                        pst = ps_s.tile([128, 128], BF16, tag="sc")
                        nc.tensor.transpose(pst, xg_b[:, c, :], ident_bf)
                        nc.vector.tensor_copy(xt[:, c * 128 : (c + 1) * 128], pst)
                    sl = slice(q * 512, (q + 1) * 512)
                    psq = ps_o.tile([128, 512], FP32, tag="out")
                    nc.tensor.matmul(psq, wd["wq"], xt[:, sl], start=True, stop=True)
                    nc.scalar.activation(qtt[:, sl], psq, ACTF.Identity, bias=bd["bq"])
                    psk = ps_o.tile([128, 512], FP32, tag="out")
                    nc.tensor.matmul(psk, wd["wk"], xt[:, sl], start=True, stop=True)
                    nc.scalar.activation(ktt[:, sl], psk, ACTF.Copy)
                xgT.append(xt)
                qt.append(qtt)
                kt.append(ktt)

            def prologue_b(i):
                # V natural [t, e] fp8: stationary = xgT chunk, moving = Wv
                vn = big.tile([128, N_CHUNKS, 128], FP8, tag="v_nat")
                for c in range(N_CHUNKS):
                    psv = ps_d.tile([128, 128], FP32, tag="den")
                    nc.tensor.matmul(
                        psv, xgT[i][:, c * 128 : (c + 1) * 128], w_bf[i]["wv"],
                        start=True, stop=True,
                    )
                    nc.vector.tensor_copy(vn[:, c, :], psv)
                v_nat.append(vn)

            prologue_a(0)

            # ---------------- attention rounds (software-pipelined) ----------
            N_TOTAL = PAIRS_PER_CORE * N_MACROS  # 16 macros across both pairs
            pt_prev = None
            for r in range(N_TOTAL + 1):
                if r == 1:
                    # V of pair 0 (needed now), then pair-1 prologue — all of
                    # it overlaps round 0's ACT exp work
                    prologue_b(0)
                    prologue_a(1)
                    prologue_b(1)
                pt_cur = [None] * N_GROUPS
                if r > 0:
                    p2, m2 = divmod(r - 1, N_MACROS)
                    ps_out = ps_o.tile([128, TQ], FP32, tag="out")
                    ps_den = ps_d.tile([128, TQ], FP32, tag="den")
                if r < N_TOTAL:
                    p1, m1 = divmod(r, N_MACROS)
                    qsl = qt[p1][:, m1 * TQ : (m1 + 1) * TQ]
                for g in range(N_GROUPS):
                    if r < N_TOTAL:
                        c0, c1 = 2 * g, 2 * g + 1
                        sc = ps_s.tile([128, 2, 512], FP32, tag="sc")
                        nc.tensor.matmul(
                            sc[:, 0, :],
                            kt[p1][:, c0 * 128 : (c0 + 1) * 128],
                            qsl, start=True, stop=True,
                        )
                        nc.tensor.matmul(
                            sc[:, 1, :],
                            kt[p1][:, c1 * 128 : (c1 + 1) * 128],
                            qsl, start=True, stop=True,
                        )
                        ptg = ptp.tile([128, 2, 512], FP8, tag="pt")
                        if DVE_EXP_G0 <= g < DVE_EXP_G0 + N_DVE_EXP:
                            # fast-exp2 on VectorE: bits = int(A*s + C), viewed fp32
                            ti = epi.tile([128, 2, 512], INT32, tag="ti")
                            nc.vector.tensor_scalar(
                                ti, sc, EXP2_A, EXP2_C,
                                mybir.AluOpType.mult, mybir.AluOpType.add,
                            )
                            nc.vector.tensor_copy(ptg, ti[:, :, :].bitcast(FP32))
                        else:
                            nc.scalar.activation(ptg, sc, ACTF.Exp, bias=exp_bias)
                        pt_cur[g] = ptg
                    if 0 < r < N_TOTAL:
                        first, last = g == 0, g == N_GROUPS - 1
                        nc.tensor.matmul(
                            ps_out,
                            v_nat[p2][:, 2 * g : 2 * g + 2, :],
                            pt_prev[g],
                            start=first, stop=last,
                            perf_mode=mybir.MatmulPerfMode.DoubleRow,
                        )
                        nc.tensor.matmul(
                            ps_den, ones_f8, pt_prev[g],
                            start=first, stop=last,
                            perf_mode=mybir.MatmulPerfMode.DoubleRow,
                        )
                if r == N_TOTAL:
                    # final round: all den MMs first so the reciprocal (the
                    # long pole of the exposed tail) overlaps the PV MMs
                    for g in range(N_GROUPS):
                        nc.tensor.matmul(
                            ps_den, ones_f8, pt_prev[g],
                            start=g == 0, stop=g == N_GROUPS - 1,
                            perf_mode=mybir.MatmulPerfMode.DoubleRow,
                        )
                    for g in range(N_GROUPS):
                        nc.tensor.matmul(
                            ps_out,
                            v_nat[p2][:, 2 * g : 2 * g + 2, :],
                            pt_prev[g],
                            start=g == 0, stop=g == N_GROUPS - 1,
                            perf_mode=mybir.MatmulPerfMode.DoubleRow,
                        )
                # epilogue of macro r-1
                if r > 0:
                    rec = epi.tile([128, TQ], FP32, tag="rec")
                    if r >= N_TOTAL - 1:
                        # tail rounds: ACT is idle there; 1/den = exp(-ln(den))
                        # keeps the slow DVE reciprocal off the exposed tail
                        dln = epi.tile([128, TQ], FP32, tag="dln")
                        nc.scalar.activation(dln, ps_den, ACTF.Ln)
                        nc.scalar.activation(rec, dln, ACTF.Exp, scale=-1.0)
                    else:
                        nc.vector.reciprocal(rec, ps_den)
                    onorm = epi.tile([128, TQ], BF16, tag="onorm")
                    nc.vector.tensor_mul(onorm, ps_out, rec)
                    nc.vector.tensor_scalar_add(onorm, onorm, b_col[p2]["bv"])
                    ofin = epi.tile([128, TQ // 128, 128], FP32, tag="ofin")
                    if r >= N_TOTAL - 1:
                        # tail rounds: PE is idle — transpose there instead of
                        # queueing 4 serialized ~1.2us xbar transposes
                        for j in range(TQ // 128):
                            pst = ps_s.tile([128, 128], BF16, tag="sc")
                            nc.tensor.transpose(
                                pst, onorm[:, j * 128 : (j + 1) * 128], ident_bf
                            )
                            nc.vector.tensor_copy(ofin[:, j, :], pst)
                    else:
                        onat = epi.tile([128, TQ // 128, 128], BF16, tag="onat")
                        for j in range(TQ // 128):
                            nc.sync.dma_start_transpose(
                                onat[:, j, :], onorm[:, j * 128 : (j + 1) * 128]
                            )
                        nc.vector.tensor_copy(ofin, onat)
                    nc.gpsimd.dma_start(
                        out=outs[p2][m2 * TQ : (m2 + 1) * TQ, :].rearrange(
                            "(c p) d -> p c d", p=128
                        ),
                        in_=ofin,
                    )
                pt_prev = pt_cur
    _split_multi_waits(nc)
    return nc


def _get_nc():
    global _NC_CACHE
    if _NC_CACHE is None:
        _NC_CACHE = build_nc()
    return _NC_CACHE


def kernel(**inputs: np.ndarray) -> np.ndarray:
    x = np.ascontiguousarray(inputs["x"], dtype=np.float32)
    Wq = np.asarray(inputs["Wq"], dtype=np.float32) * INV_SCALE
    Wk = np.asarray(inputs["Wk"], dtype=np.float32)
    Wv = np.asarray(inputs["Wv"], dtype=np.float32)
    bq = np.asarray(inputs["bq"], dtype=np.float32) * INV_SCALE
    bv = np.asarray(inputs["bv"], dtype=np.float32)

    nc = _get_nc()

    in_maps = []
    for core in range(N_CORES):
        m = {}
        for i in range(PAIRS_PER_CORE):
            pair = core * PAIRS_PER_CORE + i
            b, g = pair // G, pair % G
            sl = slice(g * GS, (g + 1) * GS)
            m[f"x{i}"] = np.ascontiguousarray(x[b, :, sl])
            m[f"wq{i}"] = np.ascontiguousarray(Wq[g])
            m[f"wk{i}"] = np.ascontiguousarray(Wk[g])
            m[f"wv{i}"] = np.ascontiguousarray(Wv[g])
            m[f"bq{i}"] = np.ascontiguousarray(bq[g].reshape(1, GS))
            m[f"bv{i}"] = np.ascontiguousarray(bv[g].reshape(1, GS))
        in_maps.append(m)

    global _LAST_IN_MAPS
    _LAST_IN_MAPS = in_maps

    from concourse.bass_utils import run_bass_kernel_spmd

    res = run_bass_kernel_spmd(nc, in_maps, list(range(N_CORES)))

    y = np.empty((B, T, F), dtype=np.float32)
    for core in range(N_CORES):
        for i in range(PAIRS_PER_CORE):
            pair = core * PAIRS_PER_CORE + i
            b, g = pair // G, pair % G
            y[b, :, g * GS : (g + 1) * GS] = res.results[core][f"y{i}"]
    return y


# revision 44
# speedup vs baseline: 1.0552x; 1.0016x over previous
"""Grouped-Query Attention kernel for Trainium2 (8 NeuronCores, SPMD).

Problem: x [4, 4096, 512] fp32, per-group Dense Q/K/V (G=4 groups of 128
features), full softmax attention within each (batch, group) pair, output
re-concatenated to [4, 4096, 512].

Sharding: B*G = 16 fully independent attention problems -> 2 per core.

v2 design (vs v1 baseline @666us):
  - 1/sqrt(gs) scale folded into Wq/bq on the host; K bias dropped
    (softmax is invariant to a per-row constant); V bias folded into the
    epilogue (rows of softmax(P) sum to 1, so O = (P V)/den + bv exactly).
  - V computed in NATURAL [t, e] layout directly (stationary = xgT chunk,
    moving = Wv) -- kills 32 PE transposes + 8 MMs per pair.
  - Q^T/K^T PSUM->SBUF evacuation moved to ScalarE (ACT is idle in the
    prologue; bias applied via ACT's free affine).
  - Attention is software-pipelined at ROUND granularity: round r issues
    score MMs of macro r interleaved (group by group) with PV/den MMs of
    macro r-1, so the PE never waits on ACT exp and stays HAM-warm.
  - exp batched as [128, 1024] ACT instructions (2 score chunks per
    instruction) to amortize ACT's 352-cycle overhead.
  - epilogue: reciprocal_approx_fast (~5x faster than DVE reciprocal),
    bias-add pre-transpose via per-partition tensor_scalar.
Compute dtype bf16 (fp32 accumulation in PSUM).
"""

import os
import sys

sys.path.insert(0, "/opt/trn_rl_repo")

import numpy as np

import concourse.bass as bass
import concourse.mybir as mybir
import concourse.tile as tile
from concourse.masks import make_identity

B, T, F, G = 4, 4096, 512, 4
GS = F // G  # 128
N_CORES = 8
PAIRS_PER_CORE = (B * G) // N_CORES  # 2
TQ = 512  # query tile width per macro (matches PSUM bank for fp32 out/den)
N_MACROS = T // TQ  # 8 per pair
N_CHUNKS = T // 128  # 32 key/time chunks
N_GROUPS = N_CHUNKS // 2  # 16 two-chunk score groups per macro
INV_SCALE = float(1.0 / (np.sqrt(np.float32(GS)) + 1e-9))

FP32 = mybir.dt.float32
BF16 = mybir.dt.bfloat16
FP8 = mybir.dt.float8e4
INT32 = mybir.dt.int32
ACTF = mybir.ActivationFunctionType
EXP_SHIFT = -4.0  # exp(s-4): keeps fp8e4m3 pt in range; cancels in normalize
# Schraudolph fast-exp2 constants: int(A*s + C) bit-viewed as fp32 ~= exp(s-4)
EXP2_A = float(2**23 * np.log2(np.e))
EXP2_C = float(2**23 * 126.94269504 + EXP_SHIFT * 2**23 * np.log2(np.e))
N_DVE_EXP = 0  # exp groups on VectorE — measured HARMFUL at every placement
# (end: delays pt for next round's PV; start: holds score-PSUM slots behind
# the prior epilogue in the DVE FIFO; mid: still 436us vs 329us). Keep 0.
DVE_EXP_G0 = 7

_NC_CACHE = None
_LAST_IN_MAPS = None


def _split_multi_waits(nc):
    """Walrus codegen rejects instructions carrying more than one semaphore
    wait on several instruction structs (DMA DIRECT2D, tensor_scalar, LDW).
    Hoist all-but-the-last wait of any multi-wait instruction onto same-engine
    NoOps inserted immediately before it: the sequencer executes them in
    order, so the gating semantics are identical."""
    n_split = 0
    for func in nc.m.functions:
        for block in func.blocks:
            new = []
            for inst in block.instructions:
                si = inst.sync_info
                waits = list(si.on_wait) if (si is not None and si.on_wait) else []
                if len(waits) > 1:
                    for w in waits[:-1]:
                        nop = mybir.InstNoOp(
                            name=nc.get_next_instruction_name(), ins=[], outs=[]
                        )
                        nop.engine = inst.engine
                        nop.sync_info = mybir.SyncInfo(on_wait=[w], on_update=[])
                        new.append(nop)
                        n_split += 1
                    inst.sync_info = mybir.SyncInfo(
                        on_wait=[waits[-1]],
                        on_update=list(si.on_update) if si.on_update else [],
                    )
                new.append(inst)
            block.instructions = new
    return n_split


def build_nc():
    nc = bass.Bass()

    ins = []
    outs = []
    for i in range(PAIRS_PER_CORE):
        ins.append(
            dict(
                x=nc.declare_dram_parameter(f"x{i}", [T, GS], FP32, isOutput=False),
                wq=nc.declare_dram_parameter(f"wq{i}", [GS, GS], FP32, isOutput=False),
                wk=nc.declare_dram_parameter(f"wk{i}", [GS, GS], FP32, isOutput=False),
                wv=nc.declare_dram_parameter(f"wv{i}", [GS, GS], FP32, isOutput=False),
                bq=nc.declare_dram_parameter(f"bq{i}", [1, GS], FP32, isOutput=False),
                bv=nc.declare_dram_parameter(f"bv{i}", [1, GS], FP32, isOutput=False),
            )
        )
        outs.append(nc.declare_dram_parameter(f"y{i}", [T, GS], FP32, isOutput=True))

    with tile.TileContext(nc) as tc:
        with (
            tc.tile_pool(name="consts", bufs=1) as consts,
            tc.tile_pool(name="wstage", bufs=2) as wstage,
            tc.tile_pool(name="xgf", bufs=2) as xgf,
            tc.tile_pool(name="xgb", bufs=2) as xgb,
            tc.tile_pool(name="big", bufs=2) as big,  # xgT/qt/kt/v_nat per pair
            tc.tile_pool(name="ptp", bufs=18) as ptp,  # exp'd prob groups
            tc.tile_pool(name="epi", bufs=2) as epi,  # epilogue sbuf tiles
            tc.tile_pool(name="ps_s", bufs=2, space="PSUM") as ps_s,  # scores
            tc.tile_pool(name="ps_o", bufs=2, space="PSUM") as ps_o,  # out^T
            tc.tile_pool(name="ps_d", bufs=2, space="PSUM") as ps_d,  # denom
        ):
            ident_bf = consts.tile([128, 128], BF16)
            make_identity(nc, ident_bf)
            ones_f8 = consts.tile([128, 2, 128], FP8)
            nc.vector.memset(ones_f8, 1.0)
            exp_bias = consts.tile([128, 1], FP32)
            nc.vector.memset(exp_bias, EXP_SHIFT)
            # preload the exp activation table during the prologue so the
            # first real exp doesn't pay the ~2.7us ACT_TABLE_LOAD
            warm = consts.tile([128, 1], FP32)
            nc.scalar.activation(warm, exp_bias, ACTF.Exp)

            w_bf = []  # per pair dict
            b_col = []
            xgT = []
            qt = []
            kt = []
            v_nat = []

            def prologue_a(i):
                """x load + transposes + Q^T/K^T, interleaved per 4-chunk slice
                so kt/qt slices land progressively and round-i scores (and
                ACT exp) can start as early as possible."""
                p = ins[i]
                xg_f = xgf.tile([128, N_CHUNKS, 128], FP32, tag="xg_f")
                xg_b = xgb.tile([128, N_CHUNKS, 128], BF16, tag="xg_b")
                xt = big.tile([128, T], BF16, tag="xgT")
                qtt = big.tile([128, T], BF16, tag="qt")
                ktt = big.tile([128, T], BF16, tag="kt")

                def load_piece(q):
                    nc.gpsimd.dma_start(
                        out=xg_f[:, q * 4 : (q + 1) * 4, :],
                        in_=p["x"][q * 512 : (q + 1) * 512, :].rearrange(
                            "(c p) d -> p c d", p=128
                        ),
                    )

                # x piece 0 ahead of the weight DMAs on the SWDGE queue — it
                # gates the whole pipeline; the weights aren't needed until
                # the first QK matmul ~2us later
                load_piece(0)
                wd = {}
                for nm in ("wq", "wk", "wv"):
                    wf = wstage.tile([128, 128], FP32, tag="wf")
                    nc.gpsimd.dma_start(out=wf, in_=p[nm][:, :])
                    wb = consts.tile([128, 128], BF16, tag=f"{nm}{i}")
                    nc.vector.tensor_copy(wb, wf)
                    wd[nm] = wb
                w_bf.append(wd)
                bd = {}
                for nm in ("bq", "bv"):
                    bc = consts.tile([128, 1], FP32, tag=f"{nm}{i}")
                    nc.gpsimd.dma_start(out=bc, in_=p[nm][:, :].rearrange("o d -> d o"))
                    bd[nm] = bc
                b_col.append(bd)

                for q in range(8):
                    qsl_c = slice(q * 4, (q + 1) * 4)
                    if q + 1 < 8:
                        load_piece(q + 1)
                    nc.vector.tensor_copy(xg_b[:, qsl_c, :], xg_f[:, qsl_c, :])
                    for c in range(q * 4, (q + 1) * 4):
                        pst = ps_s.tile([128, 128], BF16, tag="sc")
                        nc.tensor.transpose(pst, xg_b[:, c, :], ident_bf)
                        nc.vector.tensor_copy(xt[:, c * 128 : (c + 1) * 128], pst)
                    sl = slice(q * 512, (q + 1) * 512)
                    psq = ps_o.tile([128, 512], FP32, tag="out")
                    nc.tensor.matmul(psq, wd["wq"], xt[:, sl], start=True, stop=True)
                    nc.scalar.activation(qtt[:, sl], psq, ACTF.Identity, bias=bd["bq"])
                    psk = ps_o.tile([128, 512], FP32, tag="out")
                    nc.tensor.matmul(psk, wd["wk"], xt[:, sl], start=True, stop=True)
                    nc.scalar.activation(ktt[:, sl], psk, ACTF.Copy)
                xgT.append(xt)
                qt.append(qtt)
                kt.append(ktt)

            def prologue_b(i):
                # V natural [t, e] fp8: stationary = xgT chunk, moving = Wv
                vn = big.tile([128, N_CHUNKS, 128], FP8, tag="v_nat")
                for c in range(N_CHUNKS):
                    psv = ps_d.tile([128, 128], FP32, tag="den")
                    nc.tensor.matmul(
                        psv, xgT[i][:, c * 128 : (c + 1) * 128], w_bf[i]["wv"],
                        start=True, stop=True,
                    )
                    nc.vector.tensor_copy(vn[:, c, :], psv)
                v_nat.append(vn)

            prologue_a(0)

            # ---------------- attention rounds (software-pipelined) ----------
            N_TOTAL = PAIRS_PER_CORE * N_MACROS  # 16 macros across both pairs
            pt_prev = None
            for r in range(N_TOTAL + 1):
                if r == 1:
                    # V of pair 0 (needed now), then pair-1 prologue — all of
                    # it overlaps round 0's ACT exp work
                    prologue_b(0)
                    prologue_a(1)
                    prologue_b(1)
                pt_cur = [None] * N_GROUPS
                if r > 0:
                    p2, m2 = divmod(r - 1, N_MACROS)
                    ps_out = ps_o.tile([128, TQ], FP32, tag="out")
                    ps_den = ps_d.tile([128, TQ], FP32, tag="den")
                if r < N_TOTAL:
                    p1, m1 = divmod(r, N_MACROS)
                    qsl = qt[p1][:, m1 * TQ : (m1 + 1) * TQ]
                for g in range(N_GROUPS):
                    if r < N_TOTAL:
                        c0, c1 = 2 * g, 2 * g + 1
                        sc = ps_s.tile([128, 2, 512], FP32, tag="sc")
                        nc.tensor.matmul(
                            sc[:, 0, :],
                            kt[p1][:, c0 * 128 : (c0 + 1) * 128],
                            qsl, start=True, stop=True,
                        )
                        nc.tensor.matmul(
                            sc[:, 1, :],
                            kt[p1][:, c1 * 128 : (c1 + 1) * 128],
                            qsl, start=True, stop=True,
                        )
                        ptg = ptp.tile([128, 2, 512], FP8, tag="pt")
                        if DVE_EXP_G0 <= g < DVE_EXP_G0 + N_DVE_EXP:
                            # fast-exp2 on VectorE: bits = int(A*s + C), viewed fp32
                            ti = epi.tile([128, 2, 512], INT32, tag="ti")
                            nc.vector.tensor_scalar(
                                ti, sc, EXP2_A, EXP2_C,
                                mybir.AluOpType.mult, mybir.AluOpType.add,
                            )
                            nc.vector.tensor_copy(ptg, ti[:, :, :].bitcast(FP32))
                        else:
                            nc.scalar.activation(ptg, sc, ACTF.Exp, bias=exp_bias)
                        pt_cur[g] = ptg
                    if 0 < r < N_TOTAL:
                        first, last = g == 0, g == N_GROUPS - 1
                        nc.tensor.matmul(
                            ps_out,
                            v_nat[p2][:, 2 * g : 2 * g + 2, :],
                            pt_prev[g],
                            start=first, stop=last,
                            perf_mode=mybir.MatmulPerfMode.DoubleRow,
                        )
                        nc.tensor.matmul(
                            ps_den, ones_f8, pt_prev[g],
                            start=first, stop=last,
                            perf_mode=mybir.MatmulPerfMode.DoubleRow,
                        )
                if r == N_TOTAL:
                    # final round: all den MMs first so the reciprocal (the
                    # long pole of the exposed tail) overlaps the PV MMs
                    for g in range(N_GROUPS):
                        nc.tensor.matmul(
                            ps_den, ones_f8, pt_prev[g],
                            start=g == 0, stop=g == N_GROUPS - 1,
                            perf_mode=mybir.MatmulPerfMode.DoubleRow,
                        )
                    for g in range(N_GROUPS):
                        nc.tensor.matmul(
                            ps_out,
                            v_nat[p2][:, 2 * g : 2 * g + 2, :],
                            pt_prev[g],
                            start=g == 0, stop=g == N_GROUPS - 1,
                            perf_mode=mybir.MatmulPerfMode.DoubleRow,
                        )
                # epilogue of macro r-1
                if r > 0:
                    rec = epi.tile([128, TQ], FP32, tag="rec")
                    if r >= N_TOTAL - 1:
                        # tail rounds: ACT is idle there; 1/den = exp(-ln(den))
                        # keeps the slow DVE reciprocal off the exposed tail
                        dln = epi.tile([128, TQ], FP32, tag="dln")
                        nc.scalar.activation(dln, ps_den, ACTF.Ln)
                        nc.scalar.activation(rec, dln, ACTF.Exp, scale=-1.0)
                    else:
                        nc.vector.reciprocal(rec, ps_den)
                    onorm = epi.tile([128, TQ], BF16, tag="onorm")
                    nc.vector.tensor_mul(onorm, ps_out, rec)
                    nc.vector.tensor_scalar_add(onorm, onorm, b_col[p2]["bv"])
                    ofin = epi.tile([128, TQ // 128, 128], FP32, tag="ofin")
                    if r >= N_TOTAL - 1:
                        # tail rounds: PE is idle — transpose there instead of
                        # queueing 4 serialized ~1.2us xbar transposes
                        for j in range(TQ // 128):
                            pst = ps_s.tile([128, 128], BF16, tag="sc")
                            nc.tensor.transpose(
                                pst, onorm[:, j * 128 : (j + 1) * 128], ident_bf
                            )
                            nc.vector.tensor_copy(ofin[:, j, :], pst)
                    else:
                        onat = epi.tile([128, TQ // 128, 128], BF16, tag="onat")
                        for j in range(TQ // 128):
                            nc.sync.dma_start_transpose(
                                onat[:, j, :], onorm[:, j * 128 : (j + 1) * 128]
                            )
                        nc.vector.tensor_copy(ofin, onat)
                    nc.gpsimd.dma_start(
                        out=outs[p2][m2 * TQ : (m2 + 1) * TQ, :].rearrange(
                            "(c p) d -> p c d", p=128
                        ),
                        in_=ofin,
                    )
                pt_prev = pt_cur
    _split_multi_waits(nc)
    return nc


def _get_nc():
    global _NC_CACHE
    if _NC_CACHE is None:
        _NC_CACHE = build_nc()
    return _NC_CACHE


def kernel(**inputs: np.ndarray) -> np.ndarray:
    x = np.ascontiguousarray(inputs["x"], dtype=np.float32)
    Wq = np.asarray(inputs["Wq"], dtype=np.float32) * INV_SCALE
    Wk = np.asarray(inputs["Wk"], dtype=np.float32)
    Wv = np.asarray(inputs["Wv"], dtype=np.float32)
    bq = np.asarray(inputs["bq"], dtype=np.float32) * INV_SCALE
    bv = np.asarray(inputs["bv"], dtype=np.float32)

    nc = _get_nc()

    in_maps = []
    for core in range(N_CORES):
        m = {}
        for i in range(PAIRS_PER_CORE):
            pair = core * PAIRS_PER_CORE + i
            b, g = pair // G, pair % G
            sl = slice(g * GS, (g + 1) * GS)
            m[f"x{i}"] = np.ascontiguousarray(x[b, :, sl])
            m[f"wq{i}"] = np.ascontiguousarray(Wq[g])
            m[f"wk{i}"] = np.ascontiguousarray(Wk[g])
            m[f"wv{i}"] = np.ascontiguousarray(Wv[g])
            m[f"bq{i}"] = np.ascontiguousarray(bq[g].reshape(1, GS))
            m[f"bv{i}"] = np.ascontiguousarray(bv[g].reshape(1, GS))
        in_maps.append(m)

    global _LAST_IN_MAPS
    _LAST_IN_MAPS = in_maps

    from concourse.bass_utils import run_bass_kernel_spmd

    res = run_bass_kernel_spmd(nc, in_maps, list(range(N_CORES)))

    y = np.empty((B, T, F), dtype=np.float32)
    for core in range(N_CORES):
        for i in range(PAIRS_PER_CORE):
            pair = core * PAIRS_PER_CORE + i
            b, g = pair // G, pair % G
            y[b, :, g * GS : (g + 1) * GS] = res.results[core][f"y{i}"]
    return y
